# revision 40
# baseline (speedup 1.0000x reference)
"""MoE routing kernel for 8 Trainium2 NeuronCores (Bass/Tile, SPMD).

Strategy (expert-parallel, matching the sharding hint):
  - Host computes the gate (softmax + top-2) and dispatches tokens: each of
    the 8 cores owns 2 of the 16 routed experts and receives only the tokens
    routed to its experts (gathered + transposed + zero-padded).  Experts are
    assigned to cores by sorted token count: slot A holds ranks 0-7 (cap =
    largest count), slot B ranks 8-15 (cap = rank-8 count), so padding waste
    is per-tier instead of global-max.
  - The output layer (ow) commutes with the weighted combine and is folded
    into each expert's second matmul on the host (w2ot = w2[e].T @ ow.T).
  - The shared expert is sharded over its intermediate dim (2048/8=256 rows
    per core); every core computes a partial for all 2048 tokens, with ow
    folded in.  Bias terms that commute with the output layer (b2, sb2, ob)
    are applied analytically on the host.
  - All loads ride ONE consumption-ordered DMA queue (SP engine), so the
    single ~358 GB/s per-core HBM share is spent in exactly the order the
    PE needs data: xgA, w1A, sw1, xt-chunk0, sw3, w3A, xt-chunk1, w2oA,
    xgB, w1B, xt-chunk2, w3B, xt-chunk3, w2oB, sw2.
  - Stage 1 of each stream is split into a p1-phase (w1 matmuls + leaky-relu
    into a-tiles) and a p3-phase (w3 matmuls + bias-add + mul + trailing
    stage-2), matching that arrival order.
  - Elementwise chain spread across engines: leaky-relu on Scalar (ACT),
    p3+b3 on GpSimd (Pool), product on Vector (DVE).
  - Device outputs are fp16; host combines in fp32.
"""
import sys

if "/opt/trn_rl_repo" not in sys.path:
    sys.path.insert(0, "/opt/trn_rl_repo")

import numpy as np
import concourse.bass as bass
import concourse.tile as tile
from concourse import mybir
from concourse.bass_utils import run_bass_kernel_spmd

B = 2048
W = 512
E = 16
TOPK = 2
INTER = 1024
SH = 2048
OUT = 128
NCORES = 8
EPC = E // NCORES          # experts per core = 2
SHS = SH // NCORES         # shared-expert inter slice per core = 256
KW = W // 128              # k-tiles over W = 4
MI = INTER // 128          # m-tiles over INTER = 8
MS = SHS // 128            # m-tiles over shared slice = 2
SC = 512                   # shared-expert token chunk
NSC = B // SC              # shared chunks = 4
F32 = mybir.dt.float32
F16 = mybir.dt.float16
DT = F16                   # device datapath dtype for matmul operands
NPDT = np.float16

# set by test.py to collect a profile; results stashed in LAST_RESULTS
TRACE = False
TRACE_KW = {}
LAST_RESULTS = None


def _legalize_waits(nc):
    """This container's walrus accepts at most 1 sync wait per instruction
    (2 for EventSemaphore).  Hoist excess waits emitted by the Tile
    scheduler into standalone EventSemaphore instructions."""
    for fn in nc.m.functions:
        for blk in fn.blocks:
            out = []
            changed = False
            for inst in blk.instructions:
                si = getattr(inst, "sync_info", None)
                waits = list(si.on_wait) if si is not None and si.on_wait else []
                cap = 2 if isinstance(inst, mybir.InstEventSemaphore) else 1
                if len(waits) > cap:
                    extra, keep = waits[:-cap], waits[-cap:]
                    for i in range(0, len(extra), 2):
                        out.append(mybir.InstEventSemaphore(
                            name=nc.get_next_instruction_name(),
                            engine=inst.engine,
                            ins=[], outs=[],
                            sync_info=mybir.SyncInfo(
                                on_wait=list(extra[i:i + 2]), on_update=[]),
                        ))
                    si.on_wait = keep
                    changed = True
                out.append(inst)
            if changed:
                blk.instructions = out


def _token_chunks(cap):
    """Split [0, cap) into chunks of <=512 (multiples of 16)."""
    chunks = []
    off = 0
    while off < cap:
        sz = min(512, cap - off)
        chunks.append((off, sz))
        off += sz
    return chunks


def _build_nc(capA, capB, legalize=True):
    """Build the SPMD Bass program for slot token capacities capA/capB
    (multiples of 16)."""
    nc = bass.Bass("TRN2", target_bir_lowering=False, debug=False)

    def din(name, shape, dt=DT):
        return nc.dram_tensor(name, shape, dt, kind="ExternalInput").ap()

    xt = din("xt", [128, KW, B])             # x.T k-blocks: [:, k, :] = x.T[128k:128(k+1), :]
    xgA = din("xgA", [128, KW * capA])       # gathered tokens, slot A expert
    xgB = din("xgB", [128, KW * capB])
    w1A = din("w1A", [128, KW * INTER])      # w1[eA].T packed k-blocks
    w3A = din("w3A", [128, KW * INTER])
    w1B = din("w1B", [128, KW * INTER])
    w3B = din("w3B", [128, KW * INTER])
    w2oA = din("w2oA", [128, MI * OUT])      # (w2[eA].T @ ow.T) packed k-blocks
    w2oB = din("w2oB", [128, MI * OUT])
    sw1t = din("sw1t", [128, KW * SHS])      # shared slice: sw1[s].T packed
    sw3t = din("sw3t", [128, KW * SHS])
    sw2ot = din("sw2ot", [128, MS * OUT])    # (sw2[:, s].T @ ow.T) packed
    # bias cols: A:b1[8],b3[8]  B:b1[8],b3[8]  sb1[2],sb3[2]
    bias = din("bias", [128, 4 * MI + 2 * MS], F32)

    yrA = nc.dram_tensor("yrA", [128, capA], F16, kind="ExternalOutput").ap()
    yrB = nc.dram_tensor("yrB", [128, capB], F16, kind="ExternalOutput").ap()
    zt = nc.dram_tensor("zt", [128, B], F16, kind="ExternalOutput").ap()

    LR = mybir.ActivationFunctionType.Lrelu
    IDT = mybir.ActivationFunctionType.Identity

    with tile.TileContext(nc) as tc:
        with tc.tile_pool(name="wts", bufs=1) as wts, \
             tc.tile_pool(name="work", bufs=2) as work, \
             tc.tile_pool(name="ats", bufs=1) as ats, \
             tc.tile_pool(name="hts", bufs=1) as hts, \
             tc.tile_pool(name="outs", bufs=2) as outs, \
             tc.tile_pool(name="ps", bufs=2, space="PSUM") as ps:

            # ---- ACT table preload: tiny Lrelu on a dummy tile at t=0 so the
            # ~1.3us ACT_TABLE_LOAD overlaps the DMA ramp instead of blocking
            # the first real activation.
            dumm = wts.tile([128, 1], F32, tag="dumm")
            nc.gpsimd.memset(dumm[:], 0.0)
            dumo = wts.tile([128, 1], DT, tag="dumo")
            nc.scalar.activation(dumo[:], dumm[:], LR, alpha=0.01)

            # ---- PE warm-up: a train of tiny matmuls during the DMA wait so
            # the HAM clock gate is already at 2.4 GHz when real work arrives.
            import os as _os
            NWARM = int(_os.environ.get("K_WARM", "48"))
            if NWARM:
                dumw = wts.tile([128, 128], DT, tag="dumw")
                nc.gpsimd.memset(dumw[:], 0.0)
                wp = ps.tile([128, 512], F32, tag="p3", bufs=2)
                for _ in range(NWARM):
                    nc.tensor.matmul(wp[:, :64], dumw[:], dumw[:, :64],
                                     start=True, stop=True, skip_group_check=True)

            bias_t = wts.tile([128, bias.shape[1]], F32, tag="bias")
            nc.scalar.dma_start(bias_t[:], bias[:])

            # ---------------- consumption-ordered DMA stream (SP queue) -----
            slots = {"A": dict(cap=capA, bcol=0, yr=yrA),
                     "B": dict(cap=capB, bcol=2 * MI, yr=yrB)}

            def slot_dma(s, nm, dram, f, pieces=1):
                t = work.tile([128, f], DT, tag=f"{nm}{s}", bufs=1)
                q = f // pieces
                for i in range(pieces):
                    nc.sync.dma_start(t[:, i * q:(i + 1) * q],
                                      dram[:, i * q:(i + 1) * q])
                slots[s][nm] = t

            xt_ts = []

            def xt_dma(c):
                t = wts.tile([128, KW, SC], DT, tag=f"xtc{c}")
                nc.sync.dma_start(t[:], xt[:, :, c * SC:(c + 1) * SC])
                xt_ts.append(t)

            slot_dma("A", "xg", xgA, KW * capA)
            slot_dma("A", "w1", w1A, KW * INTER, pieces=4)
            sw1_t = wts.tile([128, KW * SHS], DT, tag="sw1")
            nc.sync.dma_start(sw1_t[:], sw1t[:])
            xt_dma(0)
            slot_dma("A", "w3", w3A, KW * INTER, pieces=2)
            sw3_t = wts.tile([128, KW * SHS], DT, tag="sw3")
            nc.sync.dma_start(sw3_t[:], sw3t[:])
            xt_dma(1)
            slot_dma("A", "w2o", w2oA, MI * OUT)
            slot_dma("B", "xg", xgB, KW * capB)
            slot_dma("B", "w1", w1B, KW * INTER)
            xt_dma(2)
            slot_dma("B", "w3", w3B, KW * INTER)
            xt_dma(3)
            slot_dma("B", "w2o", w2oB, MI * OUT)
            sw2_t = wts.tile([128, MS * OUT], DT, tag="sw2")
            nc.sync.dma_start(sw2_t[:], sw2ot[:])

            def b_ap(col):  # [128,1] per-partition bias column
                return bias_t[:, col:col + 1]

            # ---------------- compute generators ----------------
            def expert_steps(s):
                """Two-phase expert stream.  Phase 1: p1 matmuls + leaky-relu
                into a-tiles (needs only xg+w1).  Phase 2: p3 matmuls +
                bias-add (GpSimd) + mul (DVE) + trailing stage-2."""
                S = slots[s]
                cap, bcol = S["cap"], S["bcol"]
                xg_t, w1_t, w3_t, w2_t = S["xg"], S["w1"], S["w3"], S["w2o"]
                yr_d = S["yr"]
                chunks = _token_chunks(cap)
                nch = len(chunks)
                a_tiles = {}
                for (c0, csz) in chunks:
                    for m in range(MI):
                        p1 = ps.tile([128, csz], F32, tag="p1", bufs=4)
                        for k in range(KW):
                            nc.tensor.matmul(
                                p1[:], w1_t[:, (m * KW + k) * 128:(m * KW + k + 1) * 128],
                                xg_t[:, k * cap + c0:k * cap + c0 + csz],
                                start=(k == 0), stop=(k == KW - 1))
                        a = ats.tile([128, csz], DT, tag=f"a{s}{m}", bufs=nch)
                        nc.scalar.activation(a[:], p1[:], LR,
                                             bias=b_ap(bcol + m), alpha=0.01)
                        a_tiles[(c0, m)] = a
                        yield
                LAG = 3
                for (c0, csz) in chunks:
                    py = ps.tile([128, csz], F32, tag="py", bufs=2)
                    pend = []
                    for m in range(MI):
                        p3 = ps.tile([128, csz], F32, tag="p3", bufs=2)
                        for k in range(KW):
                            nc.tensor.matmul(
                                p3[:], w3_t[:, (m * KW + k) * 128:(m * KW + k + 1) * 128],
                                xg_t[:, k * cap + c0:k * cap + c0 + csz],
                                start=(k == 0), stop=(k == KW - 1))
                        t3 = work.tile([128, csz], DT, tag="t3", bufs=3)
                        nc.scalar.activation(t3[:], p3[:], IDT,
                                             bias=b_ap(bcol + MI + m))
                        ht = hts.tile([128, csz], DT, tag=f"ht{m}", bufs=2)
                        nc.vector.tensor_mul(ht[:], a_tiles[(c0, m)][:], t3[:])
                        pend.append((m, ht))
                        if len(pend) > LAG:
                            md, htd = pend.pop(0)
                            nc.tensor.matmul(py[:], w2_t[:, md * OUT:(md + 1) * OUT],
                                             htd[:], start=(md == 0), stop=(md == MI - 1))
                        if not (m == MI - 1 and (c0, csz) == chunks[-1]):
                            yield
                    for md, htd in pend:
                        nc.tensor.matmul(py[:], w2_t[:, md * OUT:(md + 1) * OUT],
                                         htd[:], start=(md == 0), stop=(md == MI - 1))
                    yo = outs.tile([128, csz], F16, tag="yo")
                    nc.scalar.activation(yo[:], py[:], IDT)
                    nc.sync.dma_start(yr_d[:, c0:c0 + csz], yo[:])
                yield

            def shared_steps():
                """Per chunk: p1-phase (per m), then p3-phase (per m); the
                pz stage-2 accumulation trails by one chunk."""
                pend = []

                def flush(pzg, gc0, hs_list):
                    for m, hs in hs_list:
                        nc.tensor.matmul(pzg[:], sw2_t[:, m * OUT:(m + 1) * OUT],
                                         hs[:], start=(m == 0), stop=(m == MS - 1))
                    zo = outs.tile([128, SC], F16, tag="zo")
                    nc.vector.tensor_copy(zo[:], pzg[:])
                    nc.gpsimd.dma_start(zt[:, gc0:gc0 + SC], zo[:])

                for c in range(NSC):
                    sa = {}
                    for m in range(MS):
                        p1 = ps.tile([128, SC], F32, tag="p1", bufs=4)
                        for k in range(KW):
                            nc.tensor.matmul(
                                p1[:], sw1_t[:, (m * KW + k) * 128:(m * KW + k + 1) * 128],
                                xt_ts[c][:, k, :],
                                start=(k == 0), stop=(k == KW - 1))
                        a = ats.tile([128, SC], DT, tag=f"sa{m}", bufs=2)
                        nc.scalar.activation(a[:], p1[:], LR,
                                             bias=b_ap(4 * MI + m), alpha=0.01)
                        sa[m] = a
                        yield
                    pz = ps.tile([128, SC], F32, tag="py", bufs=2)
                    hs_list = []
                    for m in range(MS):
                        p3 = ps.tile([128, SC], F32, tag="p3", bufs=2)
                        for k in range(KW):
                            nc.tensor.matmul(
                                p3[:], sw3_t[:, (m * KW + k) * 128:(m * KW + k + 1) * 128],
                                xt_ts[c][:, k, :],
                                start=(k == 0), stop=(k == KW - 1))
                        t3 = work.tile([128, SC], DT, tag="t3", bufs=3)
                        nc.vector.tensor_scalar_add(t3[:], p3[:], b_ap(4 * MI + MS + m))
                        hs = hts.tile([128, SC], DT, tag=f"hs{m}", bufs=2)
                        nc.vector.tensor_mul(hs[:], sa[m][:], t3[:])
                        hs_list.append((m, hs))
                        yield
                    pend.append((pz, c * SC, hs_list))
                    if len(pend) > 1:
                        flush(*pend.pop(0))
                for args_ in pend:
                    flush(*args_)
                yield

            # ---------------- interleave ----------------
            # Emission order sets per-engine scheduler priority.  Tuned to the
            # DMA arrival order above; tunable via K_PATTERN.
            import os as _os
            pattern = _os.environ.get(
                "K_PATTERN",
                "AAAAAAAA SS AA SS AAAA S AA S "
                "BB S BB SS BBBB S BB S "
                "SSSS SS BBBBBB")
            gens = {"A": expert_steps("A"), "B": expert_steps("B"),
                    "S": shared_steps()}
            for ch in pattern:
                if ch == " ":
                    continue
                g = gens.get(ch)
                if g is None:
                    continue
                try:
                    next(g)
                except StopIteration:
                    gens[ch] = None
            for ch in ("A", "B", "S"):
                g = gens[ch]
                if g is None:
                    continue
                for _ in g:
                    pass

    if legalize:
        _legalize_waits(nc)
    return nc


_NC_CACHE = {}


def _pack_kblocks(mat):
    """[Ktot, F] -> [128, (Ktot/128)*F] with col block k = mat[128k:128(k+1), :]."""
    ktot, f = mat.shape
    assert ktot % 128 == 0
    return np.ascontiguousarray(
        mat.reshape(ktot // 128, 128, f).transpose(1, 0, 2).reshape(128, -1))


def _pack_mkblocks(mat):
    """[Ktot, F] -> [128, (F/128)*(Ktot/128)*128] with col block (m*KW+k) =
    mat[128k:128(k+1), 128m:128(m+1)] — m-major so early m-tiles arrive first."""
    ktot, f = mat.shape
    assert ktot % 128 == 0 and f % 128 == 0
    return np.ascontiguousarray(
        mat.reshape(ktot // 128, 128, f // 128, 128)
        .transpose(1, 2, 0, 3).reshape(128, -1))


def prepare(x, task_id, gate_w, w1, b1, w2, b2, w3, b3,
            sw1, sb1, sw2, sb2, sw3, sb3, ow, ob):
    """Host-side routing + packing.  Returns everything needed to launch the
    device program and combine its partial outputs."""
    x = np.asarray(x, np.float32)
    f32 = lambda a: np.asarray(a, np.float32)
    gate_w, w1, b1, w2, b2, w3, b3 = map(f32, (gate_w, w1, b1, w2, b2, w3, b3))
    sw1, sb1, sw2, sb2, sw3, sb3, ow, ob = map(f32, (sw1, sb1, sw2, sb2, sw3, sb3, ow, ob))

    # ---- host gate: softmax + top-2 (the routing decision) ----
    logits = x @ gate_w.T
    logits -= logits.max(axis=1, keepdims=True)
    ex = np.exp(logits)
    scores = ex / ex.sum(axis=1, keepdims=True)            # [B, E] fp32
    order = np.argsort(-scores, axis=1, kind="stable")[:, :TOPK]   # [B, 2]

    tok_lists = []
    for e in range(E):
        sel = np.nonzero((order == e).any(axis=1))[0]
        tok_lists.append(sel)
    counts = np.array([len(t) for t in tok_lists])
    rank = np.argsort(-counts, kind="stable")              # experts by count desc
    pad16 = lambda n: max(128, -(-n // 16) * 16)
    capA = pad16(int(counts[rank[0]]))
    capB = pad16(int(counts[rank[NCORES]]))
    # core c owns slot A expert rank[c], slot B expert rank[2*NCORES-1-c]
    slotA = [int(rank[c]) for c in range(NCORES)]
    slotB = [int(rank[2 * NCORES - 1 - c]) for c in range(NCORES)]

    if (capA, capB) not in _NC_CACHE:
        _NC_CACHE[(capA, capB)] = _build_nc(capA, capB)
    nc = _NC_CACHE[(capA, capB)]

    # ---- pack per-core inputs (device datapath dtype) ----
    xt_p = _pack_kblocks(x.T.copy()).astype(NPDT).reshape(128, KW, B)
    in_maps = []
    for c in range(NCORES):
        im = {"xt": xt_p}
        for s, e, cap in (("A", slotA[c], capA), ("B", slotB[c], capB)):
            toks = tok_lists[e]
            xge = np.zeros((W, cap), np.float32)
            xge[:, :len(toks)] = x[toks].T
            im["xg" + s] = _pack_kblocks(xge).astype(NPDT)
            im["w1" + s] = _pack_mkblocks(w1[e].T.copy()).astype(NPDT)
            im["w3" + s] = _pack_mkblocks(w3[e].T.copy()).astype(NPDT)
            im["w2o" + s] = _pack_kblocks(w2[e].T @ ow.T).astype(NPDT)
        bias_cols = []
        for e in (slotA[c], slotB[c]):
            bias_cols.append(b1[e].reshape(MI, 128).T)     # [128, MI]
            bias_cols.append(b3[e].reshape(MI, 128).T)
        sl = slice(c * SHS, (c + 1) * SHS)
        bias_cols.append(sb1[sl].reshape(MS, 128).T)
        bias_cols.append(sb3[sl].reshape(MS, 128).T)
        im["sw1t"] = _pack_mkblocks(sw1[sl].T.copy()).astype(NPDT)
        im["sw3t"] = _pack_mkblocks(sw3[sl].T.copy()).astype(NPDT)
        im["sw2ot"] = _pack_kblocks(sw2[:, sl].T @ ow.T).astype(NPDT)
        im["bias"] = np.ascontiguousarray(np.concatenate(bias_cols, axis=1))
        in_maps.append(im)

    # dense combine weights [B, E] (zero except the top-2 experts per token)
    combine_w = np.zeros((B, E), np.float32)
    rows = np.arange(B)
    combine_w[rows[:, None], order] = np.take_along_axis(scores, order, axis=1)
    # analytic bias terms: sum_e combine[:,e] * (b2[e] @ ow.T)  +  sb2 @ ow.T + ob
    base = combine_w @ (b2 @ ow.T) + sb2 @ ow.T + ob

    return dict(nc=nc, slotA=slotA, slotB=slotB, capA=capA, capB=capB,
                in_maps=in_maps, tok_lists=tok_lists,
                combine_w=combine_w, base=base)


def combine(p, results):
    """Combine per-core device partials into the full [B, OUT] output."""
    tok_lists, combine_w = p["tok_lists"], p["combine_w"]
    out = p["base"].astype(np.float32).copy()
    for c in range(NCORES):
        r = results[c]
        out += r["zt"].astype(np.float32).T
        for s, key in (("slotA", "yrA"), ("slotB", "yrB")):
            e = p[s][c]
            toks = tok_lists[e]
            yre = r[key][:, :len(toks)].astype(np.float32)  # [OUT, cnt]
            out[toks] += combine_w[toks, e][:, None] * yre.T
    return out


def kernel(x, task_id, gate_w, w1, b1, w2, b2, w3, b3,
           sw1, sb1, sw2, sb2, sw3, sb3, ow, ob):
    global LAST_RESULTS
    p = prepare(x, task_id, gate_w, w1, b1, w2, b2, w3, b3,
                sw1, sb1, sw2, sb2, sw3, sb3, ow, ob)
    res = run_bass_kernel_spmd(
        p["nc"], p["in_maps"], core_ids=list(range(NCORES)),
        trace=TRACE, **TRACE_KW)
    LAST_RESULTS = res
    return combine(p, res.results)
